# revision 2
# baseline (speedup 1.0000x reference)
"""Trainium2 Bass kernel for BlockDecomposedSSMAttention.

Math: y = x @ W with W = B.T @ A @ C.T  (associativity collapse).

Key insight vs the replicated-W variant (which computed the full 1024x1024
W chain on all 8 cores and was purely PE-issue-bound): a COLUMN slice of W
propagates through the chain,
    W[:, S] = B.T @ (A @ (C.T[:, S])),
so sharding the output columns makes the chain cost proportional to the
slice width — no replicated 1024x1024 chain, no collectives.

Distribution (2 row-groups x 4 col-groups over 8 cores):
  core c -> rg = c // 4 (8192 rows of x), cg = c % 4 (256 cols of y/W).
  Per core PE work: chain 2 * (1024*1024*256) + main 8192*1024*256
  = 128 matmuls @ 256 moving + 256 matmuls @ 512 moving  (bf16, 1 cyc/row)
  ~= 73 us PE vs ~140 us replicated.  Measured ~90 us HW (was 145-170 us).

Rejected experimentally: fp8 DoubleRow main loop with dual residual
(x8@w8 + xr@w8 + x8@wr) — numerically fine (4.5e-3) but DoubleRow matmuls
run at ~1 cyc/row on this hw (219-226 ns/mm, not the modeled 0.5), so 384
matmuls lose to 256 bf16 ones.  Splitting input DMA issues across the
sync+scalar hw queues also regressed (+44 ns on every matmul from extra
semaphore traffic).

Everything bf16 (tolerance is 2e-2; measured numpy-emulated error ~4.5e-3):
halves DMA (24.5 MB/core) and halves LDWEIGHTS cost vs f32r.

Layouts (host marshals so every DMA is contiguous per partition):
  at:  [128 jp, 8 it, 8 jo, 128 i'] = A[it*128+i', jo*128+jp]   (A^T tiled)
  b:   [128 ip, 8 kt, 8 io, 128 k'] = B[io*128+ip, kt*128+k']
  ctc: [128 jp, 8 jo, 256 oc]       = C[cg*256+oc, jo*128+jp]   (C^T cols)
  xt:  [128 kp, 8 ko, 8192 m]       = x_rg[m, ko*128+kp]        (x^T tiled)
  yt:  [256 oc, 8192 m]             = y_rg_cg^T  (host transposes back)

Chain (per core): U[i, oc] = A @ C^T[:, S]; Wc[k, oc] = B^T @ U; then
  y^T[oc, m] = Wc^T[oc-tile] stationary x x^T moving (N=512).
Host-side work is layout marshalling + bf16 casts only; every FLOP runs on
the device.
"""

import os
import sys

import numpy as np

if "/opt/trn_rl_repo" not in sys.path:
    sys.path.insert(0, "/opt/trn_rl_repo")

BATCH, SEQ, D = 4, 4096, 1024
NCORES = 8
RG, CG = 2, 4                 # row-groups x col-groups
ROWS = BATCH * SEQ            # 16384
MSH = ROWS // RG              # 8192 rows per core
OCW = D // CG                 # 256 output cols per core
P = 128
KT = D // P                   # 8 tiles along any 1024 dim
MC = 512                      # main-loop moving chunk (m rows per psum tile)
NMC = MSH // MC               # 16 m-chunks

_CACHE: dict = {}


def _build_nc():
    import concourse.mybir as mybir
    import concourse.tile as tile
    from concourse import bacc

    f32 = mybir.dt.float32
    bf16 = mybir.dt.bfloat16

    nc = bacc.Bacc(
        "TRN2", target_bir_lowering=False, debug=False, num_devices=NCORES
    )

    # hdr packs ctc (first 2048 cols) + at tile 0 (next 1024 cols) so the
    # first matmul's dependencies arrive via a single DMA issue.
    hdr_in = nc.dram_tensor(
        "hdr_in", [P, KT * OCW + KT * P], bf16, kind="ExternalInput"
    )
    at_in = nc.dram_tensor("at_in", [P, KT - 1, KT, P], bf16, kind="ExternalInput")
    b_in = nc.dram_tensor("b_in", [P, KT, KT, P], bf16, kind="ExternalInput")
    xt_in = nc.dram_tensor("xt_in", [P, KT, MSH], bf16, kind="ExternalInput")
    yt_out = nc.dram_tensor("yt_out", [OCW, MSH], bf16, kind="ExternalOutput")

    with tile.TileContext(nc) as tc:
        with (
            tc.tile_pool(name="big", bufs=1) as big,
            tc.tile_pool(name="ycopy", bufs=6) as ycopy,
            tc.tile_pool(name="psc", bufs=2, space="PSUM") as psc,
            tc.tile_pool(name="psm", bufs=4, space="PSUM") as psm,
        ):
            # SBUF tiles
            hdr_sb = big.tile([P, KT * OCW + KT * P], bf16)
            at_sb = big.tile([P, KT - 1, KT, P], bf16)
            b_sb = big.tile([P, KT, KT, P], bf16)
            u_sb = big.tile([P, KT, OCW], bf16)
            w_sb = big.tile([P, KT, OCW], bf16)
            xt_sb = big.tile([P, KT, MSH], bf16)

            def ctc_t(jo):
                return hdr_sb[:, jo * OCW : (jo + 1) * OCW]

            def at_t(t, jo):
                if t == 0:
                    off = KT * OCW
                    return hdr_sb[:, off + jo * P : off + (jo + 1) * P]
                return at_sb[:, t - 1, jo]

            # ---- input DMAs, in first-consumption order ----
            nc.sync.dma_start(hdr_sb[:], hdr_in.ap())
            nc.sync.dma_start(at_sb[:, 0:3], at_in.ap()[:, 0:3])
            nc.sync.dma_start(at_sb[:, 3:7], at_in.ap()[:, 3:7])
            for t in range(0, KT, 4):
                nc.sync.dma_start(b_sb[:, t : t + 4], b_in.ap()[:, t : t + 4])
            for q in range(KT):
                nc.sync.dma_start(
                    xt_sb[:, :, q * 1024 : (q + 1) * 1024],
                    xt_in.ap()[:, :, q * 1024 : (q + 1) * 1024],
                )

            # ---- stage A: U[i, oc] = A @ C^T[:, S]  (contract over j) ----
            for t in range(KT):
                pu = psc.tile([P, OCW], f32)
                for jo in range(KT):
                    nc.tensor.matmul(
                        pu[:],
                        at_t(t, jo),
                        ctc_t(jo),
                        start=(jo == 0),
                        stop=(jo == KT - 1),
                    )
                nc.vector.tensor_copy(u_sb[:, t], pu[:])

            # ---- stage B: Wc[k, oc] = B^T @ U  (contract over i) ----
            for t in range(KT):
                pw = psc.tile([P, OCW], f32)
                for io in range(KT):
                    nc.tensor.matmul(
                        pw[:],
                        b_sb[:, t, io],
                        u_sb[:, io],
                        start=(io == 0),
                        stop=(io == KT - 1),
                    )
                nc.vector.tensor_copy(w_sb[:, t], pw[:])

            # ---- main: y^T[oc, m] = Wc^T @ x^T  (contract over k) ----
            for mc in range(NMC):
                for c2 in range(OCW // P):
                    pm = psm.tile([P, MC], f32)
                    for ko in range(KT):
                        nc.tensor.matmul(
                            pm[:],
                            w_sb[:, ko, c2 * P : (c2 + 1) * P],
                            xt_sb[:, ko, mc * MC : (mc + 1) * MC],
                            start=(ko == 0),
                            stop=(ko == KT - 1),
                        )
                    yt = ycopy.tile([P, MC], bf16)
                    nc.vector.tensor_copy(yt[:], pm[:])
                    nc.gpsimd.dma_start(
                        yt_out.ap()[
                            c2 * P : (c2 + 1) * P, mc * MC : (mc + 1) * MC
                        ],
                        yt[:],
                    )

    nc.compile()
    return nc


def _get_nc():
    if "nc" not in _CACHE:
        _CACHE["nc"] = _build_nc()
    return _CACHE["nc"]


def _bf16(a):
    import ml_dtypes

    return np.ascontiguousarray(np.asarray(a, dtype=ml_dtypes.bfloat16))


def _make_in_maps(x, A, B, C):
    x2 = np.asarray(x, dtype=np.float32).reshape(ROWS, D)
    A = np.asarray(A, dtype=np.float32)
    B = np.asarray(B, dtype=np.float32)
    C = np.asarray(C, dtype=np.float32)

    # at[jp, it, jo, i'] = A[it*128+i', jo*128+jp]
    at_full = _bf16(A.reshape(KT, P, KT, P).transpose(3, 0, 2, 1))
    at = np.ascontiguousarray(at_full[:, 1:])          # tiles 1..7
    at0 = at_full[:, 0].reshape(P, KT * P)             # tile 0 -> header
    # b[ip, kt, io, k'] = B[io*128+ip, kt*128+k']
    b = _bf16(B.reshape(KT, P, KT, P).transpose(1, 2, 0, 3))

    hdrs = []
    for cg in range(CG):
        # ctc[jp, jo, oc] = C[cg*256+oc, jo*128+jp]
        Cc = C[cg * OCW : (cg + 1) * OCW, :]            # [256, 1024]
        ctc = _bf16(Cc.T.reshape(KT, P, OCW).transpose(1, 0, 2))
        hdrs.append(
            np.ascontiguousarray(
                np.concatenate([ctc.reshape(P, KT * OCW), at0], axis=1)
            )
        )

    xts = []
    for rg in range(RG):
        sh = x2[rg * MSH : (rg + 1) * MSH]              # [8192, 1024]
        # xt[kp, ko, m] = sh[m, ko*128+kp]
        xts.append(_bf16(sh.T.reshape(KT, P, MSH).transpose(1, 0, 2)))

    in_maps = []
    for c in range(NCORES):
        rg, cg = c // CG, c % CG
        in_maps.append(
            {"hdr_in": hdrs[cg], "at_in": at, "b_in": b, "xt_in": xts[rg]}
        )
    return in_maps


def _install_ntff_hook():
    """The agent image's ``antenv`` lacks ``axon_hooks``; recreate it and
    register the ctypes-based NTFF profile hook so ``trace=True`` yields
    exec_time_ns."""
    import contextlib
    import ctypes
    import types

    if "antenv.axon_hooks" in sys.modules:
        return True
    so_path = "/opt/axon/libaxon_pjrt.so"
    if not os.path.exists(so_path):
        return False
    lib = ctypes.CDLL(so_path)
    if not hasattr(lib, "axon_start_nrt_profile"):
        return False
    lib.axon_start_nrt_profile.argtypes = [
        ctypes.POINTER(ctypes.c_int64),
        ctypes.c_size_t,
    ]
    lib.axon_start_nrt_profile.restype = ctypes.c_int64
    lib.axon_stop_nrt_profile.argtypes = [ctypes.c_char_p]
    lib.axon_stop_nrt_profile.restype = ctypes.c_int64

    @contextlib.contextmanager
    def _hook(output_dir, device_ids):
        import jax

        jax.devices()
        if device_ids:
            ids = (ctypes.c_int64 * len(device_ids))(*device_ids)
            rc = lib.axon_start_nrt_profile(ids, len(device_ids))
        else:
            rc = lib.axon_start_nrt_profile(None, 0)
        if rc != 0:
            raise RuntimeError(f"axon_start_nrt_profile rc={rc}")
        try:
            yield
        finally:
            n = lib.axon_stop_nrt_profile(str(output_dir).encode())
            print(f"ntff profile: {n} file(s) written to {output_dir}")

    mod = types.ModuleType("antenv.axon_hooks")
    _state = {"hook": _hook}
    mod.set_axon_ntff_profile_hook = lambda h: _state.__setitem__("hook", h)
    mod.get_axon_ntff_profile_hook = lambda: _state["hook"]
    sys.modules["antenv.axon_hooks"] = mod
    import antenv

    antenv.axon_hooks = mod
    return True


def run(x, A, B, C, trace=False):
    """Run on hardware; returns (y_full, exec_time_ns_or_None)."""
    from concourse import bass_utils
    from concourse.bass_interp import get_hw_module

    if trace and not _install_ntff_hook():
        trace = False
    if trace:
        if not getattr(bass_utils.upload_artifacts, "_safe", False):
            _orig_upload = bass_utils.upload_artifacts

            def _safe_upload(tmpdir):
                try:
                    return _orig_upload(tmpdir)
                except Exception as e:
                    print(f"upload_artifacts skipped ({type(e).__name__}): {e}")
                    return str(tmpdir)

            _safe_upload._safe = True
            bass_utils.upload_artifacts = _safe_upload

    nc = _get_nc()
    in_maps = _make_in_maps(x, A, B, C)

    old_m = nc.m
    nc.m = get_hw_module(nc.m)
    try:
        res = bass_utils.run_bass_kernel_spmd(
            nc, in_maps, core_ids=list(range(NCORES)), trace=trace
        )
    finally:
        nc.m = old_m

    y = np.empty((ROWS, D), dtype=np.float32)
    for c in range(NCORES):
        rg, cg = c // CG, c % CG
        yt = np.asarray(res.results[c]["yt_out"])       # [256, 8192] bf16
        y[rg * MSH : (rg + 1) * MSH, cg * OCW : (cg + 1) * OCW] = (
            yt.astype(np.float32).T
        )
    return y.reshape(BATCH, SEQ, D), res.exec_time_ns


def kernel(x, A, B, C):
    y, _ = run(x, A, B, C, trace=False)
    return y
